# revision 1
# baseline (speedup 1.0000x reference)
"""CombinedRotaryEmbedding Trainium2 kernel (fp16 I/O, host-pretransposed).

Math (per 64-dim head, position s):
    y   = x @ R2            R2 = [R_even | R_odd]  ->  y = [u(32) | v(32)]
    out = [u*cos - v*sin | u*sin + v*cos]          cos/sin = f(position, freq)

Strategy (8-way data parallel over the sequence dim, 512 positions/core):
  - host: compose R (tiny [64,64]); build blockdiag([R2,R2]) in fp16; cast +
    pre-transpose x to fp16 with features on partitions so the device needs
    no transpose pass; build per-core compact cos/sin tables
    T[pos%128, blk, 0] = [cos|-sin], T[.,.,1] = [sin|cos] (64 wide, fp16).
  - device, per core (16 row tiles of 128 rows x 1024 feats):
      DMA : fp16 in/out, contiguous 2-4KB/partition transfers, all issued
            from SP (in-DMA for d+1 emitted before out-DMA of d so an
            out-DMA's sequencer wait never delays input prefetch)
      PE  : 8 fp16 matmuls [128x128x128] per row tile (y2 into PSUM fp32),
            4 matmuls packed per 2KB PSUM bank
      ACT : copy PSUM fp32 -> SBUF fp16 (2 contiguous copies per row tile)
      DVE : fused mul t12[t] = y * T[t] (one op, free=2048, fp16 2x mode)
      DVE/GPSIMD: crossed add out = t12_u + t12_v via strided APs (one op,
            free=1024); slow Pool add on j=0, fast DVE add last on j=1 so
            the out-DMA's final dependency resolves early
  - ramp/tail: warm-up ops preload the ACT function table and keep PE's
    p-state counter running; the first double tile is processed as two
    single row tiles; the last tile's adds run on DVE in halves with
    half-size stores.
  - Elementwise tables fold the [u|v] -> [lo|hi] pairing so a single add
    finishes the rotary: t12[0] = [u*c | -v*s], t12[1] = [u*s | v*c];
    out_lo = t12[0,u]+t12[0,v], out_hi = t12[1,u]+t12[1,v].
"""

import numpy as np

import concourse.bacc as bacc
import concourse.bass as bass
import concourse.tile as tile
from concourse import mybir
from concourse.bass_utils import run_bass_kernel_spmd

N_CORES = 8
B, S, N_STATE = 4, 4096, 1024
H, D = 16, 64           # heads, head dim
HALF = D // 2           # 32 rotary freqs
S_SH = S // N_CORES     # 512 positions per core
ROWS = B * S_SH         # 2048 rows of [1024] per core
RT = ROWS // 128        # 16 row tiles
DBL = RT // 2           # 8 double row tiles (DMA granularity)
CBLK = S_SH // 128      # 4 distinct position blocks per core
CW = 128 + CBLK * 2 * D  # combined const width (r2 | ccss)
F32 = mybir.dt.float32
F16 = mybir.dt.float16

_compiled = {}


# add-engine assignment: Pool takes all j=0 adds, plus these j=1 row tiles
# (early ones while DVE ramps, rt11 to smooth the late out-DMA flow); DVE
# keeps the rest so each tile pair's final add resolves fast
POOL_ADD_CUT = 4
POOL_EXTRA = (11,)
SPLIT0 = True   # process the first two row tiles in half-tile steps
RAMP_POOL_COPY = ()  # ramp tiles whose h1 copy runs on the Pool (no gain measured)
DVE_FORCE = ()  # row tiles forced onto DVE despite the Pool rule
POOL_COPY_RT = ()  # mid-stream row tiles whose h1 copy runs on the Pool
LAST_J0_SPLIT = False
SPLIT_OUT_D = (3, 4, 5)  # mid tiles whose store ships per row tile
ADD_SPLIT_D = ()  # mid tiles with halved Pool j0 add + quarter stores
DEFER_D6 = False  # issue out(6) after out(7,j0) on the SP sequencer
POOL_SET = None  # explicit Pool add set (row tiles 2..13); None = rule above


def _build_nc():
    nc = bacc.Bacc("TRN2")
    # x pre-transposed+tiled on host: [d, p=feat%128, j=rt%2, g=chunk, r=row]
    x_in = nc.dram_tensor("x", [DBL, 128, 2, 8, 128], F16, kind="ExternalInput")
    # x0r = [blockdiag(R2,R2) | x tile (0,0)]: one contiguous head DMA
    x0r_in = nc.dram_tensor("x0r", [128, 128 + 1024], F16, kind="ExternalInput")
    # cst = ccss tables; ccss[p, blk*128 + t*64 + f]:
    # t=0 -> [cos|-sin], t=1 -> [sin|cos] for position blk*128+p
    cst_in = nc.dram_tensor("cst", [128, CW - 128], F16, kind="ExternalInput")
    # out[d, p=row%128, j, col]; host un-permutes
    out_d = nc.dram_tensor("out", [DBL, 128, 2, N_STATE], F16,
                           kind="ExternalOutput")

    with tile.TileContext(nc) as tc:
        with (
            tc.tile_pool(name="const", bufs=1) as const,
            tc.tile_pool(name="xin", bufs=7) as xin,
            tc.tile_pool(name="xin0", bufs=2) as xin0,
            tc.tile_pool(name="ypsum", bufs=8, space="PSUM") as ypsum,
            tc.tile_pool(name="yfp", bufs=6) as yfp,
            tc.tile_pool(name="t12p", bufs=7) as t12p,
            tc.tile_pool(name="outp", bufs=7) as outp,
        ):
            cst_sb = const.tile([128, CW - 128], F16)
            cst_a = cst_sb[:]
            x0r = const.tile([128, 128 + 1024], F16)
            r2_a = x0r[:, 0:128]

            # warm-ups: preload the ACT function table and start PE's p-state
            # clock while the first DMAs are in flight
            warm = const.tile([128, 1], F16)
            nc.vector.memset(warm[:], 0.0)
            nc.scalar.copy(out=warm[:], in_=warm[:])
            wpsum = ypsum.tile([128, 4, 128], F32, tag="yp")
            nc.tensor.matmul(wpsum[0:1, 0, 0:1], warm[:], warm[:],
                             start=True, stop=True)

            x_tiles = {}

            def prefetch(d):
                x_t = xin.tile([128, 2, 8, 128], F16, tag="x")
                nc.sync.dma_start(out=x_t[:], in_=x_in[d])
                x_tiles[d] = x_t

            def rowtile(xchunks, rt, og, split=False, copy_eng=None):
                """xchunks: [128, 8, 128] AP, or a pair of [128, 4, 128]
                APs (one per half); og: out [128, 1024].  split=True runs the
                fused mul per half right after its copy (shorter ramp).
                copy_eng optionally overrides the engine per half-copy."""
                if copy_eng is None:
                    copy_eng = (nc.scalar, nc.scalar)
                if isinstance(xchunks, tuple):
                    halves = xchunks
                    getchunk = lambda g: halves[g // 4][:, g % 4, :]
                else:
                    getchunk = lambda g: xchunks[:, g, :]
                c = rt % CBLK
                yf = yfp.tile([128, N_STATE], F16, tag="yf")
                t12 = t12p.tile([128, 2, N_STATE], F16, tag="t12")
                t12w = t12[:]

                def mul(h0, h1):
                    yfa = yf[:]
                    nh = h1 - h0
                    nc.vector.tensor_mul(
                        bass.AP(tensor=t12w.tensor,
                                offset=t12w.offset + h0 * D,
                                ap=[list(t12w.ap[0]), [N_STATE, 2], [D, nh],
                                    [1, D]]),
                        bass.AP(tensor=yfa.tensor, offset=yfa.offset + h0 * D,
                                ap=[list(yfa.ap[0]), [0, 2], [D, nh], [1, D]]),
                        bass.AP(tensor=cst_a.tensor,
                                offset=cst_a.offset + c * 2 * D,
                                ap=[list(cst_a.ap[0]), [D, 2], [0, nh],
                                    [1, D]]),
                    )

                for h in range(2):
                    # 4 matmuls pack one 2KB PSUM bank (verified OK on this
                    # stack); the copy then drains a contiguous [4,128] tile
                    yp = ypsum.tile([128, 4, 128], F32, tag="yp")
                    for q in range(4):
                        g = h * 4 + q
                        nc.tensor.matmul(
                            yp[:, q, :], getchunk(g), r2_a,
                            start=True, stop=True,
                        )
                    ce = copy_eng[h]
                    if ce is nc.scalar:
                        ce.copy(out=yf[:, h * 512:(h + 1) * 512], in_=yp[:])
                    else:
                        ce.tensor_copy(yf[:, h * 512:(h + 1) * 512], yp[:])
                    if split:
                        mul(h * 8, h * 8 + 8)
                if not split:
                    mul(0, H)
                # crossed add out[t, hd, t*32:...] = t12[t, hd, u] + t12[t, hd, v]
                t12a = t12[:]

                def add(eng, f0, f1):
                    n = (f1 - f0) // D
                    o_ap = bass.AP(tensor=og.tensor, offset=og.offset + f0,
                                   ap=[list(og.ap[0]), [HALF, 2], [D, n],
                                       [1, HALF]])
                    u_ap = bass.AP(tensor=t12a.tensor, offset=t12a.offset + f0,
                                   ap=[list(t12a.ap[0]), [N_STATE, 2], [D, n],
                                       [1, HALF]])
                    v_ap = bass.AP(tensor=t12a.tensor,
                                   offset=t12a.offset + f0 + HALF,
                                   ap=[list(t12a.ap[0]), [N_STATE, 2], [D, n],
                                       [1, HALF]])
                    eng.tensor_tensor(out=o_ap, in0=u_ap, in1=v_ap,
                                      op=mybir.AluOpType.add)
                return add

            # head order: tiny r2 -> first input tile -> cos/sin tables ->
            # remaining inputs.  Everything is prefetched up front: input flow
            # never waits on the out-DMAs' sequencer stalls, and SBUF has room
            # for all of x.
            nc.sync.dma_start(out=x0r[:], in_=x0r_in[:])
            x_tiles[(0, 0)] = x0r[:, 128:1152].rearrange("p (g r) -> p g r", g=8)
            x0b = xin0.tile([128, 8, 128], F16, tag="x0")
            nc.sync.dma_start(out=x0b[:], in_=x_in[0, :, 1])
            x_tiles[(0, 1)] = x0b[:]
            nc.sync.dma_start(out=cst_sb[:], in_=cst_in[:])
            for d in range(1, DBL):
                prefetch(d)

            # first double tile: two single row tiles for a shorter ramp
            for j in range(2):
                out_t = outp.tile([128, N_STATE], F16, tag="o0")
                # h1 copies of the ramp tiles go to the (idle) Pool so the
                # ACT copy stream never starves DVE during startup
                ce = (nc.scalar, nc.gpsimd) if j in RAMP_POOL_COPY else None
                add = rowtile(x_tiles.pop((0, j)), j, out_t[:], split=SPLIT0,
                              copy_eng=ce)
                pool0 = (j in POOL_SET) if POOL_SET is not None else (j == 0)
                add(nc.gpsimd if pool0 else nc.vector, 0, N_STATE)
                nc.sync.dma_start(out=out_d[0, :, j], in_=out_t[:])

            deferred_store = None
            for d in range(1, DBL):
                x_t = x_tiles.pop(d)
                out_t = outp.tile([128, 2, N_STATE], F16, tag="o")
                last = d == DBL - 1
                for j in range(2):
                    rt_j = d * 2 + j
                    ce = ((nc.scalar, nc.gpsimd)
                          if rt_j in POOL_COPY_RT else None)
                    add = rowtile(x_t[:, j], rt_j, out_t[:, j, :],
                                  copy_eng=ce)
                    if not last:
                        # Pool takes j=0 adds (plus early j=1 while DVE ramps);
                        # DVE closes each tile pair so out-DMAs resolve fast
                        rt_i = d * 2 + j
                        if POOL_SET is not None:
                            pool = rt_i in POOL_SET
                        else:
                            pool = ((j == 0 or rt_i < POOL_ADD_CUT
                                     or rt_i in POOL_EXTRA)
                                    and rt_i not in DVE_FORCE)
                        add(nc.gpsimd if pool else nc.vector, 0, N_STATE)
                    elif j == 0:
                        if DEFER_D6 and deferred_store is not None:
                            pass  # emitted after this j0 store below
                        if LAST_J0_SPLIT:
                            # Pool (idle by now) takes half; DVE's stream
                            # shrinks by the difference
                            add(nc.gpsimd, 0, N_STATE // 2)
                            add(nc.vector, N_STATE // 2, N_STATE)
                        else:
                            add(nc.vector, 0, N_STATE)
                        nc.sync.dma_start(out=out_d[d, :, 0], in_=out_t[:, 0, :])
                        if DEFER_D6 and deferred_store is not None:
                            dd, dt = deferred_store
                            nc.sync.dma_start(out=out_d[dd], in_=dt[:])
                            deferred_store = None
                    else:
                        # final adds in halves with half-size stores: the tail
                        # transfer after the last add is only 512 cols
                        add(nc.vector, 0, N_STATE // 2)
                        nc.sync.dma_start(out=out_d[d, :, 1, 0:512],
                                          in_=out_t[:, 1, 0:512])
                        add(nc.vector, N_STATE // 2, N_STATE)
                        nc.sync.dma_start(out=out_d[d, :, 1, 512:1024],
                                          in_=out_t[:, 1, 512:1024])
                if not last:
                    if DEFER_D6 and d == DBL - 2:
                        deferred_store = (d, out_t)
                    elif d in ADD_SPLIT_D:
                        nc.sync.dma_start(out=out_d[d, :, 1],
                                          in_=out_t[:, 1, :])
                    elif d in SPLIT_OUT_D:
                        nc.sync.dma_start(out=out_d[d, :, 0],
                                          in_=out_t[:, 0, :])
                        nc.sync.dma_start(out=out_d[d, :, 1],
                                          in_=out_t[:, 1, :])
                    else:
                        nc.sync.dma_start(out=out_d[d], in_=out_t[:])
    nc.compile()
    return nc


def _compose_r2(thetas, rotation_pairs, theta_scale, rotation_matrix):
    """Replicates reference._compose_rotation, then permutes cols to [even|odd]."""
    idx = rotation_pairs.astype(np.int32)
    th = (thetas.astype(np.float32) * np.float32(theta_scale[0]))
    R = np.eye(D, dtype=np.float32)
    for k in range(th.shape[0]):
        i, j = int(idx[k, 0]), int(idx[k, 1])
        ck, sk = np.float32(np.cos(th[k])), np.float32(np.sin(th[k]))
        G = np.eye(D, dtype=np.float32)
        G[i, i] = ck
        G[i, j] = -sk
        G[j, i] = sk
        G[j, j] = ck
        R = (R @ G).astype(np.float32)
    R = (R @ rotation_matrix.astype(np.float32)).astype(np.float32)
    return np.ascontiguousarray(
        np.concatenate([R[:, 0::2], R[:, 1::2]], axis=1), dtype=np.float32
    )


def make_in_maps(x, thetas, rotation_pairs, theta_scale, rotation_matrix,
                 inv_freq):
    x = np.asarray(x, dtype=np.float32)
    r2s = _compose_r2(
        np.asarray(thetas, np.float32),
        np.asarray(rotation_pairs, np.float32),
        np.asarray(theta_scale, np.float32),
        np.asarray(rotation_matrix, np.float32),
    )
    r2 = np.zeros((128, 128), dtype=np.float32)
    r2[0:D, 0:D] = r2s
    r2[D:128, D:128] = r2s

    pos = np.arange(S, dtype=np.float32)
    sinusoid = pos[:, None] * np.asarray(inv_freq, np.float32)[None, :]  # [S,32]
    cosf = np.cos(sinusoid).astype(np.float32)
    sinf = np.sin(sinusoid).astype(np.float32)

    in_maps = []
    for k in range(N_CORES):
        blk = slice(k * S_SH, (k + 1) * S_SH)
        cb, sb = cosf[blk], sinf[blk]                       # [512, 32]
        t0 = np.concatenate([cb, -sb], axis=1)              # [512, 64]
        t1 = np.concatenate([sb, cb], axis=1)
        ccss = np.stack([t0, t1], axis=1)                   # [512, 2, 64]
        ccss = ccss.reshape(CBLK, 128, 2 * D).transpose(1, 0, 2)
        cst = np.ascontiguousarray(
            ccss.reshape(128, CBLK * 2 * D), dtype=np.float16)

        xs = x[:, blk, :].reshape(B, CBLK, 128, 8, 128)     # [b, sblk, r, g, p]
        xs = xs.transpose(0, 1, 4, 3, 2).reshape(DBL, 2, 128, 8, 128)
        xs = np.ascontiguousarray(
            xs.transpose(0, 2, 1, 3, 4), dtype=np.float16)  # [d, p, j, g, r]
        x0r = np.concatenate(
            [r2.astype(np.float16), xs[0, :, 0].reshape(128, 1024)], axis=1)
        in_maps.append({"x": xs, "x0r": np.ascontiguousarray(x0r), "cst": cst})
    return in_maps


def kernel(x, thetas, rotation_pairs, theta_scale, rotation_matrix, inv_freq):
    in_maps = make_in_maps(x, thetas, rotation_pairs, theta_scale,
                           rotation_matrix, inv_freq)
    if "nc" not in _compiled:
        _compiled["nc"] = _build_nc()
    res = run_bass_kernel_spmd(_compiled["nc"], in_maps, list(range(N_CORES))).results

    out = np.empty((B, S, N_STATE), dtype=np.float32)
    for k in range(N_CORES):
        blk = slice(k * S_SH, (k + 1) * S_SH)
        o = res[k]["out"]                                   # [d, p, j, col] f16
        o = o.transpose(0, 2, 1, 3).reshape(B, S_SH, N_STATE)
        out[:, blk, :] = o.astype(np.float32)
    return out



# revision 2
# speedup vs baseline: 1.0143x; 1.0143x over previous
"""CombinedRotaryEmbedding Trainium2 kernel (fp16 I/O, host-pretransposed).

Math (per 64-dim head, position s):
    y   = x @ R2            R2 = [R_even | R_odd]  ->  y = [u(32) | v(32)]
    out = [u*cos - v*sin | u*sin + v*cos]          cos/sin = f(position, freq)

Strategy (8-way data parallel over the sequence dim, 512 positions/core):
  - host: compose R (tiny [64,64]); build blockdiag([R2,R2]) in fp16; cast +
    pre-transpose x to fp16 with features on partitions so the device needs
    no transpose pass; build per-core compact cos/sin tables
    T[pos%128, blk, 0] = [cos|-sin], T[.,.,1] = [sin|cos] (64 wide, fp16).
  - device, per core (16 row tiles of 128 rows x 1024 feats):
      DMA : fp16 in/out, contiguous 2-4KB/partition transfers, all issued
            from SP (in-DMA for d+1 emitted before out-DMA of d so an
            out-DMA's sequencer wait never delays input prefetch)
      PE  : 8 fp16 matmuls [128x128x128] per row tile (y2 into PSUM fp32),
            4 matmuls packed per 2KB PSUM bank
      ACT : copy PSUM fp32 -> SBUF fp16 (2 contiguous copies per row tile)
      DVE : fused mul t12[t] = y * T[t] (one op, free=2048, fp16 2x mode)
      DVE/GPSIMD: crossed add out = t12_u + t12_v via strided APs (one op,
            free=1024); slow Pool add on j=0, fast DVE add last on j=1 so
            the out-DMA's final dependency resolves early
  - ramp/tail: warm-up ops preload the ACT function table and keep PE's
    p-state counter running; the first double tile is processed as two
    single row tiles; the last tile's adds run on DVE in halves with
    half-size stores.
  - Elementwise tables fold the [u|v] -> [lo|hi] pairing so a single add
    finishes the rotary: t12[0] = [u*c | -v*s], t12[1] = [u*s | v*c];
    out_lo = t12[0,u]+t12[0,v], out_hi = t12[1,u]+t12[1,v].
"""

import numpy as np

import concourse.bacc as bacc
import concourse.bass as bass
import concourse.tile as tile
from concourse import mybir
from concourse.bass_utils import run_bass_kernel_spmd

N_CORES = 8
B, S, N_STATE = 4, 4096, 1024
H, D = 16, 64           # heads, head dim
HALF = D // 2           # 32 rotary freqs
S_SH = S // N_CORES     # 512 positions per core
ROWS = B * S_SH         # 2048 rows of [1024] per core
RT = ROWS // 128        # 16 row tiles
DBL = RT // 2           # 8 double row tiles (DMA granularity)
CBLK = S_SH // 128      # 4 distinct position blocks per core
CW = 128 + CBLK * 2 * D  # combined const width (r2 | ccss)
F32 = mybir.dt.float32
F16 = mybir.dt.float16

_compiled = {}


# add-engine assignment: Pool takes all j=0 adds, plus these j=1 row tiles
# (early ones while DVE ramps, rt11 to smooth the late out-DMA flow); DVE
# keeps the rest so each tile pair's final add resolves fast
POOL_ADD_CUT = 4
POOL_EXTRA = (11,)
SPLIT0 = True   # process the first two row tiles in half-tile steps
RAMP_POOL_COPY = ()  # ramp tiles whose h1 copy runs on the Pool (no gain measured)
DVE_FORCE = ()  # row tiles forced onto DVE despite the Pool rule
POOL_COPY_RT = ()  # mid-stream row tiles whose h1 copy runs on the Pool
LAST_J0_SPLIT = False
SPLIT_OUT_D = (3, 4)  # mid tiles whose store ships per row tile
ADD_SPLIT_D = ()  # mid tiles with halved Pool j0 add + quarter stores
DEFER_D6 = False  # issue out(6) after out(7,j0) on the SP sequencer
J1_FIRST_D = (6,)  # doubles whose j=1 out ships before j=0 (j1 add on DVE is
                 # ready earlier than j0's Pool add)
POOL_SET = frozenset({0, 2, 3, 5, 6, 8, 10, 11, 13})  # explicit Pool add set


def _build_nc():
    nc = bacc.Bacc("TRN2")
    # x pre-transposed+tiled on host: [d, p=feat%128, j=rt%2, g=chunk, r=row]
    x_in = nc.dram_tensor("x", [DBL, 128, 2, 8, 128], F16, kind="ExternalInput")
    # x0r = [blockdiag(R2,R2) | x tile (0,0)]: one contiguous head DMA
    x0r_in = nc.dram_tensor("x0r", [128, 128 + 1024], F16, kind="ExternalInput")
    # cst = ccss tables; ccss[p, blk*128 + t*64 + f]:
    # t=0 -> [cos|-sin], t=1 -> [sin|cos] for position blk*128+p
    cst_in = nc.dram_tensor("cst", [128, CW - 128], F16, kind="ExternalInput")
    # out[d, p=row%128, j, col]; host un-permutes
    out_d = nc.dram_tensor("out", [DBL, 128, 2, N_STATE], F16,
                           kind="ExternalOutput")

    with tile.TileContext(nc) as tc:
        with (
            tc.tile_pool(name="const", bufs=1) as const,
            tc.tile_pool(name="xin", bufs=7) as xin,
            tc.tile_pool(name="xin0", bufs=2) as xin0,
            tc.tile_pool(name="ypsum", bufs=8, space="PSUM") as ypsum,
            tc.tile_pool(name="yfp", bufs=6) as yfp,
            tc.tile_pool(name="t12p", bufs=7) as t12p,
            tc.tile_pool(name="outp", bufs=7) as outp,
        ):
            cst_sb = const.tile([128, CW - 128], F16)
            cst_a = cst_sb[:]
            x0r = const.tile([128, 128 + 1024], F16)
            r2_a = x0r[:, 0:128]

            # warm-ups: preload the ACT function table and start PE's p-state
            # clock while the first DMAs are in flight
            warm = const.tile([128, 1], F16)
            nc.vector.memset(warm[:], 0.0)
            nc.scalar.copy(out=warm[:], in_=warm[:])
            wpsum = ypsum.tile([128, 4, 128], F32, tag="yp")
            nc.tensor.matmul(wpsum[0:1, 0, 0:1], warm[:], warm[:],
                             start=True, stop=True)

            x_tiles = {}

            def prefetch(d):
                x_t = xin.tile([128, 2, 8, 128], F16, tag="x")
                nc.sync.dma_start(out=x_t[:], in_=x_in[d])
                x_tiles[d] = x_t

            def rowtile(xchunks, rt, og, split=False, copy_eng=None):
                """xchunks: [128, 8, 128] AP, or a pair of [128, 4, 128]
                APs (one per half); og: out [128, 1024].  split=True runs the
                fused mul per half right after its copy (shorter ramp).
                copy_eng optionally overrides the engine per half-copy."""
                if copy_eng is None:
                    copy_eng = (nc.scalar, nc.scalar)
                if isinstance(xchunks, tuple):
                    halves = xchunks
                    getchunk = lambda g: halves[g // 4][:, g % 4, :]
                else:
                    getchunk = lambda g: xchunks[:, g, :]
                c = rt % CBLK
                yf = yfp.tile([128, N_STATE], F16, tag="yf")
                t12 = t12p.tile([128, 2, N_STATE], F16, tag="t12")
                t12w = t12[:]

                def mul(h0, h1):
                    yfa = yf[:]
                    nh = h1 - h0
                    nc.vector.tensor_mul(
                        bass.AP(tensor=t12w.tensor,
                                offset=t12w.offset + h0 * D,
                                ap=[list(t12w.ap[0]), [N_STATE, 2], [D, nh],
                                    [1, D]]),
                        bass.AP(tensor=yfa.tensor, offset=yfa.offset + h0 * D,
                                ap=[list(yfa.ap[0]), [0, 2], [D, nh], [1, D]]),
                        bass.AP(tensor=cst_a.tensor,
                                offset=cst_a.offset + c * 2 * D,
                                ap=[list(cst_a.ap[0]), [D, 2], [0, nh],
                                    [1, D]]),
                    )

                for h in range(2):
                    # 4 matmuls pack one 2KB PSUM bank (verified OK on this
                    # stack); the copy then drains a contiguous [4,128] tile
                    yp = ypsum.tile([128, 4, 128], F32, tag="yp")
                    for q in range(4):
                        g = h * 4 + q
                        nc.tensor.matmul(
                            yp[:, q, :], getchunk(g), r2_a,
                            start=True, stop=True,
                        )
                    ce = copy_eng[h]
                    if ce is nc.scalar:
                        ce.copy(out=yf[:, h * 512:(h + 1) * 512], in_=yp[:])
                    else:
                        ce.tensor_copy(yf[:, h * 512:(h + 1) * 512], yp[:])
                    if split:
                        mul(h * 8, h * 8 + 8)
                if not split:
                    mul(0, H)
                # crossed add out[t, hd, t*32:...] = t12[t, hd, u] + t12[t, hd, v]
                t12a = t12[:]

                def add(eng, f0, f1):
                    n = (f1 - f0) // D
                    o_ap = bass.AP(tensor=og.tensor, offset=og.offset + f0,
                                   ap=[list(og.ap[0]), [HALF, 2], [D, n],
                                       [1, HALF]])
                    u_ap = bass.AP(tensor=t12a.tensor, offset=t12a.offset + f0,
                                   ap=[list(t12a.ap[0]), [N_STATE, 2], [D, n],
                                       [1, HALF]])
                    v_ap = bass.AP(tensor=t12a.tensor,
                                   offset=t12a.offset + f0 + HALF,
                                   ap=[list(t12a.ap[0]), [N_STATE, 2], [D, n],
                                       [1, HALF]])
                    eng.tensor_tensor(out=o_ap, in0=u_ap, in1=v_ap,
                                      op=mybir.AluOpType.add)
                return add

            # head order: tiny r2 -> first input tile -> cos/sin tables ->
            # remaining inputs.  Everything is prefetched up front: input flow
            # never waits on the out-DMAs' sequencer stalls, and SBUF has room
            # for all of x.
            nc.sync.dma_start(out=x0r[:], in_=x0r_in[:])
            x_tiles[(0, 0)] = x0r[:, 128:1152].rearrange("p (g r) -> p g r", g=8)
            x0b = xin0.tile([128, 8, 128], F16, tag="x0")
            nc.sync.dma_start(out=x0b[:], in_=x_in[0, :, 1])
            x_tiles[(0, 1)] = x0b[:]
            nc.sync.dma_start(out=cst_sb[:], in_=cst_in[:])
            for d in range(1, DBL):
                prefetch(d)

            # first double tile: two single row tiles for a shorter ramp
            for j in range(2):
                out_t = outp.tile([128, N_STATE], F16, tag="o0")
                # h1 copies of the ramp tiles go to the (idle) Pool so the
                # ACT copy stream never starves DVE during startup
                ce = (nc.scalar, nc.gpsimd) if j in RAMP_POOL_COPY else None
                add = rowtile(x_tiles.pop((0, j)), j, out_t[:], split=SPLIT0,
                              copy_eng=ce)
                pool0 = (j in POOL_SET) if POOL_SET is not None else (j == 0)
                add(nc.gpsimd if pool0 else nc.vector, 0, N_STATE)
                nc.sync.dma_start(out=out_d[0, :, j], in_=out_t[:])

            deferred_store = None
            for d in range(1, DBL):
                x_t = x_tiles.pop(d)
                out_t = outp.tile([128, 2, N_STATE], F16, tag="o")
                last = d == DBL - 1
                for j in range(2):
                    rt_j = d * 2 + j
                    ce = ((nc.scalar, nc.gpsimd)
                          if rt_j in POOL_COPY_RT else None)
                    add = rowtile(x_t[:, j], rt_j, out_t[:, j, :],
                                  copy_eng=ce)
                    if not last:
                        # Pool takes j=0 adds (plus early j=1 while DVE ramps);
                        # DVE closes each tile pair so out-DMAs resolve fast
                        rt_i = d * 2 + j
                        if POOL_SET is not None:
                            pool = rt_i in POOL_SET
                        else:
                            pool = ((j == 0 or rt_i < POOL_ADD_CUT
                                     or rt_i in POOL_EXTRA)
                                    and rt_i not in DVE_FORCE)
                        add(nc.gpsimd if pool else nc.vector, 0, N_STATE)
                    elif j == 0:
                        if DEFER_D6 and deferred_store is not None:
                            pass  # emitted after this j0 store below
                        if LAST_J0_SPLIT:
                            # Pool (idle by now) takes half; DVE's stream
                            # shrinks by the difference
                            add(nc.gpsimd, 0, N_STATE // 2)
                            add(nc.vector, N_STATE // 2, N_STATE)
                        else:
                            add(nc.vector, 0, N_STATE)
                        nc.sync.dma_start(out=out_d[d, :, 0], in_=out_t[:, 0, :])
                        if DEFER_D6 and deferred_store is not None:
                            dd, dt = deferred_store
                            nc.sync.dma_start(out=out_d[dd], in_=dt[:])
                            deferred_store = None
                    else:
                        # final adds in halves with half-size stores: the tail
                        # transfer after the last add is only 512 cols; the
                        # second store issues from the (idle) ACT sequencer so
                        # the two issue chains overlap
                        add(nc.vector, 0, N_STATE // 2)
                        nc.sync.dma_start(out=out_d[d, :, 1, 0:512],
                                          in_=out_t[:, 1, 0:512])
                        add(nc.vector, N_STATE // 2, N_STATE)
                        nc.scalar.dma_start(out=out_d[d, :, 1, 512:1024],
                                            in_=out_t[:, 1, 512:1024])
                if not last:
                    if DEFER_D6 and d == DBL - 2:
                        deferred_store = (d, out_t)
                    elif d in ADD_SPLIT_D:
                        nc.sync.dma_start(out=out_d[d, :, 1],
                                          in_=out_t[:, 1, :])
                    elif d in J1_FIRST_D:
                        nc.sync.dma_start(out=out_d[d, :, 1],
                                          in_=out_t[:, 1, :])
                        nc.sync.dma_start(out=out_d[d, :, 0],
                                          in_=out_t[:, 0, :])
                    elif d in SPLIT_OUT_D:
                        nc.sync.dma_start(out=out_d[d, :, 0],
                                          in_=out_t[:, 0, :])
                        nc.sync.dma_start(out=out_d[d, :, 1],
                                          in_=out_t[:, 1, :])
                    else:
                        nc.sync.dma_start(out=out_d[d], in_=out_t[:])
    nc.compile()
    return nc


def _compose_r2(thetas, rotation_pairs, theta_scale, rotation_matrix):
    """Replicates reference._compose_rotation, then permutes cols to [even|odd]."""
    idx = rotation_pairs.astype(np.int32)
    th = (thetas.astype(np.float32) * np.float32(theta_scale[0]))
    R = np.eye(D, dtype=np.float32)
    for k in range(th.shape[0]):
        i, j = int(idx[k, 0]), int(idx[k, 1])
        ck, sk = np.float32(np.cos(th[k])), np.float32(np.sin(th[k]))
        G = np.eye(D, dtype=np.float32)
        G[i, i] = ck
        G[i, j] = -sk
        G[j, i] = sk
        G[j, j] = ck
        R = (R @ G).astype(np.float32)
    R = (R @ rotation_matrix.astype(np.float32)).astype(np.float32)
    return np.ascontiguousarray(
        np.concatenate([R[:, 0::2], R[:, 1::2]], axis=1), dtype=np.float32
    )


def make_in_maps(x, thetas, rotation_pairs, theta_scale, rotation_matrix,
                 inv_freq):
    x = np.asarray(x, dtype=np.float32)
    r2s = _compose_r2(
        np.asarray(thetas, np.float32),
        np.asarray(rotation_pairs, np.float32),
        np.asarray(theta_scale, np.float32),
        np.asarray(rotation_matrix, np.float32),
    )
    r2 = np.zeros((128, 128), dtype=np.float32)
    r2[0:D, 0:D] = r2s
    r2[D:128, D:128] = r2s

    pos = np.arange(S, dtype=np.float32)
    sinusoid = pos[:, None] * np.asarray(inv_freq, np.float32)[None, :]  # [S,32]
    cosf = np.cos(sinusoid).astype(np.float32)
    sinf = np.sin(sinusoid).astype(np.float32)

    in_maps = []
    for k in range(N_CORES):
        blk = slice(k * S_SH, (k + 1) * S_SH)
        cb, sb = cosf[blk], sinf[blk]                       # [512, 32]
        t0 = np.concatenate([cb, -sb], axis=1)              # [512, 64]
        t1 = np.concatenate([sb, cb], axis=1)
        ccss = np.stack([t0, t1], axis=1)                   # [512, 2, 64]
        ccss = ccss.reshape(CBLK, 128, 2 * D).transpose(1, 0, 2)
        cst = np.ascontiguousarray(
            ccss.reshape(128, CBLK * 2 * D), dtype=np.float16)

        xs = x[:, blk, :].reshape(B, CBLK, 128, 8, 128)     # [b, sblk, r, g, p]
        xs = xs.transpose(0, 1, 4, 3, 2).reshape(DBL, 2, 128, 8, 128)
        xs = np.ascontiguousarray(
            xs.transpose(0, 2, 1, 3, 4), dtype=np.float16)  # [d, p, j, g, r]
        x0r = np.concatenate(
            [r2.astype(np.float16), xs[0, :, 0].reshape(128, 1024)], axis=1)
        in_maps.append({"x": xs, "x0r": np.ascontiguousarray(x0r), "cst": cst})
    return in_maps


def kernel(x, thetas, rotation_pairs, theta_scale, rotation_matrix, inv_freq):
    in_maps = make_in_maps(x, thetas, rotation_pairs, theta_scale,
                           rotation_matrix, inv_freq)
    if "nc" not in _compiled:
        _compiled["nc"] = _build_nc()
    res = run_bass_kernel_spmd(_compiled["nc"], in_maps, list(range(N_CORES))).results

    out = np.empty((B, S, N_STATE), dtype=np.float32)
    for k in range(N_CORES):
        blk = slice(k * S_SH, (k + 1) * S_SH)
        o = res[k]["out"]                                   # [d, p, j, col] f16
        o = o.transpose(0, 2, 1, 3).reshape(B, S_SH, N_STATE)
        out[:, blk, :] = o.astype(np.float32)
    return out

